# revision 42
# baseline (speedup 1.0000x reference)
"""Contrastive loss on 8 Trainium2 NeuronCores (Bass/Tile).

loss * n = sum_ij [ same_ij * (s<1)(1-s) + (1-same_ij) * (s>0.3) * s ],
s = <x_i, x_j>.

Decomposition used here:
    loss * n = sum_ij relu(s) + sum_{same_ij} [relu(1-s) - relu(s)] - eps,
    eps = sum_{~same, 0<s<=0.3} s  (~1.5e-4 relative; dropped).

The same-label correction is exact and touches only ~0.2% of pairs
(labels repeat ~8x), so the HOST computes it in fp64 from the same
fp8-quantized embeddings the device consumes.  The DEVICE computes only
sum_ij relu(s_ij): one elementwise relu + row-accumulate pass over S.

Hardware constraints that shape the kernel (all verified on hw):
  * Only Act and DVE can read PSUM (GPSIMD and DMA are rejected by the
    BIR verifier), and they run at ~1 elem/cycle/lane (Act 1.2GHz with
    a 187ns accumulator-read per accum op, DVE 0.96GHz), overlapping
    each other and the PE.  Draining S is therefore the wall at ~20us,
    just above the PE's ~18us (fp8 DoubleRow streams 512 cols/chunk at
    1 col/cycle @2.4GHz plus ldweights).
  * In-place PSUM drains (read+write same bank) run ~2x slower - drains
    write their relu image to small SBUF scratch tiles instead.
  * PSUM is one manually-managed [128, 4096] arena (8 banks) split into
    four 2-bank regions: Act drains regions 0/2 alternately, DVE 1/3,
    so each engine double-buffers while PE refills freed banks.

Work split per core: 17 blocks (512x512) of the upper triangle of the
16x16 block grid.  Diagonal blocks are trimmed to their upper-triangle
columns per 128-row stripe (weight 2; the host subtracts the
double-counted diagonal 128x128 chunks).  34 drain jobs of up to 1024
cols: 17 Act (incl. the 4 trimmed diag jobs) + 17 DVE, quotas balancing
measured busy time.  Matmul in fp8e4m3 DoubleRow (K=256 in one pass).
Host: fp64 weighted column sums + corrections, divide by n.
"""

import numpy as np
import ml_dtypes

import concourse.bass as bass
import concourse.mybir as mybir
from concourse import bacc
import concourse.tile as tile
from concourse.bass_utils import run_bass_kernel_spmd

N_TOTAL = 8192
D = 256
N_CORES = 8
GB = 512                      # grid block width
G = N_TOTAL // GB             # 16 col/row blocks
NS = 17                       # task slots per core
ST = 4                        # 128-row stripes per block
MARGIN = 0.3
F32 = mybir.dt.float32
BF16 = mybir.dt.bfloat16
FP8 = mybir.dt.float8e4

BANKS = 8                     # PSUM banks of 512 f32 per partition

def make_jobs():
    """Job list: (engine, [(st, slot, c0, w), ...] chunks, weight).

    Chunks are (stripe, slot, col-start, width) sub-blocks of S.  The 8
    diagonal-block stripes (slots 0/16) are trimmed to their
    upper-triangle columns [st*128, 512) and enter at weight 2 like
    everything else; the host subtracts the double-counted diagonal
    128x128 chunks (sum_full = 2*sum_upper - sum_diagchunks).  Each trim
    job pairs the same stripe of both diagonal blocks so its two windows
    share one width.

    Measured drain costs (cayman errata): Act (172+FD)/1.2 + 187
    accum-read per op, DVE (120+FD)/0.96; both engines overlap and read
    PSUM independently.  1024-wide jobs on both, split 17/17 so busy
    times balance (~592 vs ~596 ns per 512-chunk), each engine
    double-buffering two 2-bank regions of the 8-bank PSUM ring.
    """
    jobs = []
    for st in range(ST):
        w = GB - st * 128
        jobs.append(("A", [(st, 0, st * 128, w), (st, 16, st * 128, w)],
                     2.0))
    pool = [(st, slot, 0, GB) for slot in range(1, 16) for st in range(ST)]
    assert len(pool) == 60
    it = iter(pool)
    for _ in range(13):
        jobs.append(("A", [next(it), next(it)], 2.0))
    for _ in range(17):
        jobs.append(("V", [next(it), next(it)], 2.0))
    assert next(it, None) is None
    return jobs


def plan_schedule(jobs):
    """Strict A/V alternation over four 2-bank PSUM regions.

    Act uses regions 0/2 (banks 0-1, 4-5) alternately, DVE regions 1/3
    (banks 2-3, 6-7); each engine double-buffers its own two regions so
    drains run back-to-back while PE refills freed banks.
    Returns [(job_index, bank_offset), ...].
    """
    A = [i for i, j in enumerate(jobs) if j[0] == "A"]
    V = [i for i, j in enumerate(jobs) if j[0] == "V"]
    assert len(A) == len(V) == 17
    order = []
    for k in range(17):
        order.append((A[k], 0 if k % 2 == 0 else 4))
        order.append((V[k], 2 if k % 2 == 0 else 6))
    return order


def build_program(repeats=1, ablate=frozenset()):
    """ablate (timing experiments only, breaks math): 'nocopy' drop
    drains, 'nomm' drop matmuls."""
    nc = bacc.Bacc()
    LW = NS * GB                # 8704 cols in lhs/rhs tensors
    lhs_d = nc.dram_tensor("lhs8", [128, 2, LW], FP8, kind="ExternalInput")
    rhs_d = nc.dram_tensor("rhs8", [128, 2, LW], FP8, kind="ExternalInput")

    jobs = make_jobs()
    order = plan_schedule(jobs)
    CD = len(jobs)              # one accumulator column per job
    out_d = nc.dram_tensor("out", [128, CD], F32, kind="ExternalOutput")

    AL = mybir.AluOpType
    ACT = mybir.ActivationFunctionType
    DR = mybir.MatmulPerfMode.DoubleRow

    with tile.TileContext(nc) as tc:
        with (
            tc.tile_pool(name="resident", bufs=1) as rpool,
            tc.tile_pool(name="psum", bufs=1, space="PSUM") as ppool,
        ):
            lhs8 = rpool.tile([128, 2, LW], FP8, name="lhs8")
            rhs8 = rpool.tile([128, 2, LW], FP8, name="rhs8")
            for chunk in range(4):
                sl = slice(chunk * (LW // 4), (chunk + 1) * (LW // 4))
                nc.sync.dma_start(out=lhs8[:, :, sl], in_=lhs_d[:, :, sl])
                nc.sync.dma_start(out=rhs8[:, :, sl], in_=rhs_d[:, :, sl])

            arena = ppool.tile([128, BANKS * 512], F32, name="arena")
            if "nomm" in ablate:
                nc.vector.memset(arena[:], 0.5)
            jpoolA = rpool.tile([128, 2, 1024], BF16, name="jA")
            jpoolV = rpool.tile([128, 2, 1024], BF16, name="jV")
            accD = rpool.tile([128, CD], F32, name="accD")
            nc.vector.memset(accD[:], 0.0)
            acc_ap = lambda col: accD[:, col:col + 1]

            def mm(dst, st, slot, c0, w):
                nc.tensor.matmul(
                    dst,
                    lhs8[:, :, slot * GB + st * 128: slot * GB + (st + 1) * 128],
                    rhs8[:, :, slot * GB + c0: slot * GB + c0 + w],
                    start=True, stop=True, perf_mode=DR,
                )

            def body():
                ecount = {"A": 0, "V": 0}
                for ji, off in order:
                    eng, chunks, jw = jobs[ji]
                    if "allA" in ablate:
                        eng = "A"
                    elif "allV" in ablate:
                        eng = "V"
                    if "nomm" not in ablate:
                        for h, (st, slot, c0, w) in enumerate(chunks):
                            dst = arena[:, (off + h) * 512:
                                        (off + h) * 512 + w]
                            mm(dst, st, slot, c0, w)
                    if "nocopy" in ablate:
                        continue
                    w = chunks[0][3]
                    nch = len(chunks)
                    if w == GB:
                        Tv = arena[:, off * 512: off * 512 + nch * GB]
                        out_sl = lambda j, h_: j[:, h_, 0:nch * GB]
                    else:
                        # trimmed diag job: [128, nch, w] windows at
                        # bank starts
                        X = arena[:, off * 512:(off + nch) * 512]
                        Tv = X.rearrange(
                            "p (b q) -> p b q", b=nch)[:, :, 0:w]
                        out_sl = lambda j, h_: j[:, h_, 0:nch * w] \
                            .rearrange("p (b q) -> p b q", b=nch)
                    half = ecount[eng] % 2
                    ecount[eng] += 1
                    if eng == "A":
                        nc.scalar.activation(
                            out=out_sl(jpoolA, half), in_=Tv,
                            func=ACT.Relu, bias=0.0, scale=1.0,
                            accum_out=acc_ap(ji),
                        )
                    else:
                        nc.vector.tensor_scalar(
                            out=out_sl(jpoolV, half), in0=Tv,
                            scalar1=0.0, scalar2=None,
                            op0=AL.max, op1=AL.add,
                            accum_out=acc_ap(ji),
                        )

            import contextlib
            loop_cm = tc.For_i(0, repeats, 1) if repeats > 1 else \
                contextlib.nullcontext()
            with loop_cm:
                body()

            nc.sync.dma_start(out=out_d[:], in_=accD[:])

    meta = dict(CD=CD, weights=[j[2] for j in jobs])
    return nc, meta


def host_reduce(out_arr, lin=None, meta=None):
    """[128, CD] f32 relu-sum columns -> fp64 weighted partial.

    (lin is unused with the full-f32 relu drains; kept for interface
    stability.)"""
    if meta is None:
        weights = [j[2] for j in make_jobs()]
    else:
        weights = meta["weights"]
    a = out_arr.astype(np.float64)
    tot = 0.0
    for col, w in enumerate(weights):
        tot += w * a[:, col].sum()
    return tot


def linear_sums(Xs8, jobs):
    """Exact fp64 sum_ij s_ij per (core, job) from fp8-rounded X.

    sum over a chunk region = <row-stripe sum, col-window sum>; built
    from the 64 per-stripe column sums of X."""
    SS = Xs8.reshape(G * ST, 128, -1).sum(axis=1)      # [64, D]
    lin = np.zeros((N_CORES, len(jobs)))
    for c in range(N_CORES):
        slots = task_slots(c)
        for ji, (eng, chunks, w) in enumerate(jobs):
            tot = 0.0
            for st, slot, c0, wd in chunks:
                r, j = slots[slot]
                rv = SS[r * ST + st]
                g0 = j * ST + c0 // 128
                cv = SS[g0: j * ST + ST].sum(axis=0) if wd != GB \
                    else SS[j * ST: j * ST + ST].sum(axis=0)
                tot += float(rv @ cv)
            lin[c, ji] = tot
    return lin


def task_slots(c):
    """Slot -> (row block, col block) for core c. Slots 0/16 diagonal."""
    rA, rB = c, (G - 1) - c
    blocks = [(rA, j) for j in range(rA, G)] + \
             [(rB, j) for j in range(rB, G)]
    slots = {0: (rA, rA), 16: (rB, rB)}
    fixed = set(slots.values())
    rest = [blk for blk in blocks if blk not in fixed]
    free = [s for s in range(NS) if s not in slots]
    for s, blk in zip(free, rest, strict=True):
        slots[s] = blk
    return slots


def band_correction(Xs8, ts_):
    """fp64 sum over same-label pairs of relu(1-s) - relu(s), from the
    label-sorted fp8-quantized embeddings (matches device s closely)."""
    bounds = np.flatnonzero(
        np.concatenate(([True], ts_[1:] != ts_[:-1], [True])))
    corr = 0.0
    for a, b in zip(bounds[:-1], bounds[1:]):
        Xg = Xs8[a:b]
        Sg = Xg @ Xg.T
        corr += (np.maximum(1.0 - Sg, 0.0) - np.maximum(Sg, 0.0)).sum()
    return corr


def diag_chunk_sum(Xs8):
    """fp64 sum of relu(s) over the 64 diagonal 128x128 chunks of S.

    The device drains diagonal blocks at weight 2 over their
    upper-triangle columns; sum_fullblock = 2*sum_upper - this."""
    X3 = Xs8.astype(np.float32).reshape(-1, 128, Xs8.shape[1])
    S = np.einsum("bik,bjk->bij", X3, X3, optimize=True)
    return float(np.maximum(S, 0.0).sum(dtype=np.float64))


def prepare_inputs(inputs, targets):
    X = np.asarray(inputs, dtype=np.float32)
    t = np.asarray(targets).astype(np.int64).reshape(-1)
    n, d = X.shape
    assert (n, d) == (N_TOTAL, D), f"kernel hardcoded for {N_TOTAL}x{D}"
    perm = np.argsort(t, kind="stable")
    ts_ = t[perm]
    XT = np.ascontiguousarray(X[perm].T).astype(ml_dtypes.float8_e4m3)
    # [128, 2, N]: partition lane p holds dims p (k0) and 128+p (k1)
    XK = XT.reshape(2, 128, N_TOTAL).transpose(1, 0, 2)

    in_maps = []
    for c in range(N_CORES):
        slots = task_slots(c)
        lhs = np.zeros((128, 2, NS * GB), dtype=XK.dtype)
        rhs = np.zeros((128, 2, NS * GB), dtype=XK.dtype)
        for s in range(NS):
            r, j = slots[s]
            lhs[:, :, s * GB:(s + 1) * GB] = XK[:, :, r * GB:(r + 1) * GB]
            rhs[:, :, s * GB:(s + 1) * GB] = XK[:, :, j * GB:(j + 1) * GB]
        in_maps.append({"lhs8": lhs, "rhs8": rhs})

    Xs8 = XT.T.astype(np.float64)      # [N, D] fp8-rounded, label-sorted
    corr = band_correction(Xs8, ts_) - diag_chunk_sum(Xs8)
    lin = linear_sums(Xs8, make_jobs())
    return in_maps, (corr, lin)


def run(inputs, targets, trace=False):
    in_maps, (corr, lin) = prepare_inputs(inputs, targets)
    nc, meta = build_program()
    nc.finalize()
    res = run_bass_kernel_spmd(
        nc, in_maps, core_ids=list(range(N_CORES)), trace=trace
    )
    total = corr
    for c, r in enumerate(res.results):
        total += host_reduce(r["out"], lin[c], meta)
    return np.asarray(total / N_TOTAL, dtype=np.float32), res


def kernel(inputs, targets):
    val, _ = run(inputs, targets, trace=False)
    return val
